# revision 43
# baseline (speedup 1.0000x reference)
"""Trainium2 Bass kernel for nn_DeChunkLayerReference.

The reference collapses mathematically: with state dim n=1, C==1, B=p and
per-(b,t) scalars shared across all heads, the SSD is a per-channel scalar
EMA along the M=2048 compressed sequence:

    y[b,t,:] = exp(-dt[t]) * y[b,t-1,:] + (p[t]/dt[t]) * hidden[b,t,:]

followed by a gather that duplicates each compressed row to the L=4096
output positions (plug = cumsum(boundary_mask)-1).

Closed form: y[t] = sum_{s<=t} exp(cumA[t]-cumA[s]) * w[s] * hidden[s]
with cumA = cumsum(-dt), w = p/dt.  Since dt ~ Exp(1), the decay kernel
underflows fp32 after a couple hundred steps, so y is computed with
chunked (128) lower-triangular matmuls over a few bands of chunks:

    LT_block[s,t] = exp(cumA[t] - cumA[s] + log w[s])     (masked s>t on diag)
    y_chunk_i     = sum_bands LT_block(j,i).T @ hidden_chunk_j      (PSUM acc)

The LT coefficient blocks depend only on the tiny boundary_prob /
boundary_mask inputs, so they are computed on the host in float64 and
shipped as bf16 (33 blocks ~ 1 MiB/core).  hidden is shipped bf16 in the
exact SBUF tile layout (linear DMA), the matmuls run bf16 (f32 PSUM), and
the compressed (M, qw) result is returned bf16; the host does the rep-2
plug duplication and the f32 upcast.  Per-core HBM traffic is ~5 MiB vs
the ~12.6 MiB of the all-f32 device-side variant.

Sharding over the 8 cores: (batch b in {0,1}) x (d_model quarter q in
{0..3}); each core processes its full sequence for a 512-wide channel
slice, so there is no cross-core communication at all.
"""

import numpy as np
import ml_dtypes

import concourse.tile as tile
from concourse import bacc, mybir
from concourse.bass_utils import run_bass_kernel_spmd

# Problem shapes (hardcoded per harness contract).
B = 2
M = 2048
D_MODEL = 2048
LFULL = 4096
CHUNK = 128
C = M // CHUNK          # 16 chunks
NCORES = 8
NQ = 4                  # d_model quarters
QW = D_MODEL // NQ      # 512 channels per core
EPS = 1e-4
UFLOW = -103.0          # ln(smallest fp32 denormal) ~ -103.28

GROUP = 4               # chunks per wide x tile / lt tile
NG = C // GROUP         # 4 groups
PAIR = 2                # chunks per output staging tile / DMA

F32 = mybir.dt.float32
BF16 = mybir.dt.bfloat16
NP_BF16 = ml_dtypes.bfloat16

_prog_cache: dict = {}


def _host_precompute(boundary_mask, boundary_prob):
    """float64 coefficient prep from the small inputs."""
    bm = np.asarray(boundary_mask)
    bp = np.asarray(boundary_prob)
    p = np.clip(bp[..., -1].astype(np.float32), EPS, 1.0 - EPS)
    token_idx = np.arange(bm.shape[1])[None, :] + (~bm).astype(np.int32) * bm.shape[1]
    order = np.argsort(token_idx, axis=1, kind="stable")
    p_sel = np.take_along_axis(p, order[:, :M], axis=1).astype(np.float64)  # (B, M)
    dt = -np.log1p(-p_sel)
    w = p_sel / dt
    logw = np.log(w)
    cumA = np.cumsum(-dt, axis=1)                       # (B, M) inclusive
    plug = np.cumsum(bm.astype(np.int64), axis=1) - 1   # (B, L)
    return logw, cumA, plug


def _decide_bands(cumA, logw):
    """Bands per chunk (union over batches so the SPMD program is shared)."""
    nb = []
    for i in range(C):
        T0 = i * CHUNK
        n = 1
        for bandk in range(1, i + 1):
            S0 = (i - bandk) * CHUNK
            mx = max(
                (cumA[b, T0] - cumA[b, S0:S0 + CHUNK] + logw[b, S0:S0 + CHUNK]).max()
                for b in range(cumA.shape[0])
            )
            if mx > UFLOW:
                n = bandk + 1
            else:
                break
        nb.append(n)
    return tuple(nb)


def _build_lt(nbands, cumA, logw):
    """All LT blocks, bf16, laid out [128, TOTB*128] per batch.

    Block order matches the device loop: chunks ascending, bands from
    farthest (k = nb-1) to the diagonal (k = 0).
    """
    totb = sum(nbands)
    lt = np.empty((B, CHUNK, totb * CHUNK), NP_BF16)
    smask = np.arange(CHUNK)[:, None] > np.arange(CHUNK)[None, :]  # s > t
    for b in range(B):
        pos = 0
        for i in range(C):
            T0 = i * CHUNK
            for bandk in range(nbands[i] - 1, -1, -1):
                S0 = (i - bandk) * CHUNK
                arg = (cumA[b, T0:T0 + CHUNK][None, :]
                       - cumA[b, S0:S0 + CHUNK][:, None]
                       + logw[b, S0:S0 + CHUNK][:, None])
                blk = np.exp(arg)
                if bandk == 0:
                    blk = np.where(smask, 0.0, blk)
                lt[b, :, pos * CHUNK:(pos + 1) * CHUNK] = blk.astype(NP_BF16)
                pos += 1
    return lt


def _build_program(nbands):
    P = [0]
    for nb in nbands:
        P.append(P[-1] + nb)
    totb = P[-1]

    nc = bacc.Bacc(
        "TRN2", target_bir_lowering=False, debug=False, num_devices=NCORES
    )
    x = nc.dram_tensor("x", [NG * CHUNK, GROUP * QW], BF16, kind="ExternalInput")
    ltd = nc.dram_tensor("lt", [CHUNK, totb * CHUNK], BF16, kind="ExternalInput")
    y = nc.dram_tensor("y", [CHUNK, C * QW], BF16, kind="ExternalOutput")

    with tile.TileContext(nc) as tc:
        with tc.tile_pool(name="xp", bufs=1) as xp, \
             tc.tile_pool(name="ltp", bufs=1) as ltp, \
             tc.tile_pool(name="wp", bufs=1) as wp, \
             tc.tile_pool(name="yp", bufs=6) as yp, \
             tc.tile_pool(name="wpsp", bufs=1, space="PSUM") as wpsp, \
             tc.tile_pool(name="psp", bufs=3, space="PSUM") as psp:

            warm = wp.tile([CHUNK, QW], BF16, tag="warm")
            nc.gpsimd.memset(warm[:], 0.0)

            # Inputs in compute order — chunk 0 in its own tiny tiles
            # (tile-granular readiness: the first matmul waits on two small
            # DMAs only), then chunk groups.  x segments go out on the sync
            # HWDGE ring, lt blocks on the scalar ring; both streams are
            # linear in DRAM (host pre-layout) so every partition line is
            # one contiguous descriptor.
            xin = x.rearrange("(g p) d -> g p d", p=CHUNK)

            def xslice(c0, c1):
                """DRAM AP for x chunks [c0, c1) — contiguous per group."""
                g, a = divmod(c0, GROUP)
                return xin[g][:, a * QW:(a + c1 - c0) * QW]

            xsegs = [(0, 1), (1, 4), (4, 8), (8, 12), (12, 16)]
            xtile = {}
            for c0, c1 in xsegs:
                t = xp.tile([CHUNK, (c1 - c0) * QW], BF16, tag=f"x{c0}")
                nc.sync.dma_start(out=t[:], in_=xslice(c0, c1))
                for c in range(c0, c1):
                    xtile[c] = (t, c - c0)

            lttile = {}
            for c0, c1 in xsegs:
                nbs = P[c1] - P[c0]
                t = ltp.tile([CHUNK, nbs * CHUNK], BF16, tag=f"lt{c0}")
                nc.scalar.dma_start(
                    out=t[:], in_=ltd[:, P[c0] * CHUNK:P[c1] * CHUNK])
                for c in range(c0, c1):
                    lttile[c] = (t, P[c] - P[c0])

            # PE clock warm-up: matmuls on a zeroed tile into a scratch PSUM
            # buffer while the first inputs are in flight, so the real
            # matmul stream runs at full clock from the start.
            wps = wpsp.tile([CHUNK, QW], F32, tag="wps")
            for _ in range(8):
                nc.tensor.matmul(wps[:], lhsT=warm[:, :CHUNK],
                                 rhs=warm[:], start=True, stop=True)

            def xview(j):
                t, a = xtile[j]
                return t[:, a * QW:(a + 1) * QW]

            def ltview(i, idx):
                t, base = lttile[i]
                off = (base + idx) * CHUNK
                return t[:, off:off + CHUNK]

            ACT_PAIRS = (1, 3, 5)   # pair-casts on the scalar engine
            NPAIR = C // PAIR
            for h in range(NPAIR):
                ypair = yp.tile([CHUNK, PAIR * QW], BF16, tag="yb")
                ps = psp.tile([CHUNK, PAIR * QW], F32, tag="ps")
                for ci in range(PAIR):
                    i = h * PAIR + ci
                    nb = nbands[i]
                    for idx, bandk in enumerate(range(nb - 1, -1, -1)):
                        nc.tensor.matmul(
                            ps[:, ci * QW:(ci + 1) * QW],
                            lhsT=ltview(i, idx),
                            rhs=xview(i - bandk),
                            start=(idx == 0), stop=(idx == nb - 1),
                        )
                    if i in (0, 3):
                        # Bridge the two early input waits (chunk 1-3 and
                        # group 1) with dummy matmuls so the PE clock never
                        # drops; both windows are input-blocked anyway.
                        for _ in range(4):
                            nc.tensor.matmul(wps[:], lhsT=warm[:, :CHUNK],
                                             rhs=warm[:], start=True, stop=True)
                if h == NPAIR - 1:
                    # Last pair: per-chunk casts (chunk 14's overlaps chunk
                    # 15's matmuls) and two smaller output DMAs, to shorten
                    # the tail.
                    nc.vector.tensor_copy(ypair[:, :QW], ps[:, :QW])
                    nc.vector.tensor_copy(ypair[:, QW:], ps[:, QW:])
                    nc.sync.dma_start(
                        out=y[:, (C - 2) * QW:(C - 1) * QW],
                        in_=ypair[:, :QW],
                    )
                    nc.sync.dma_start(
                        out=y[:, (C - 1) * QW:C * QW],
                        in_=ypair[:, QW:],
                    )
                else:
                    if h in ACT_PAIRS:
                        nc.scalar.copy(ypair[:], ps[:])
                    else:
                        nc.vector.tensor_copy(ypair[:], ps[:])
                    nc.sync.dma_start(
                        out=y[:, h * PAIR * QW:(h + 1) * PAIR * QW],
                        in_=ypair[:],
                    )
            # Trailing dummies: keep the tensor stream alive past the last
            # real matmul so its end-of-stream drain doesn't delay the
            # completion signals the final casts wait on.
            for _ in range(6):
                nc.tensor.matmul(wps[:], lhsT=warm[:, :CHUNK],
                                 rhs=warm[:], start=True, stop=True)
    nc.compile()
    return nc


def _run(inputs, trace=False):
    hidden = np.asarray(inputs["hidden_states"], dtype=np.float32)
    logw, cumA, plug = _host_precompute(inputs["boundary_mask"],
                                        inputs["boundary_prob"])

    rep = LFULL // M
    fast = np.array_equal(
        plug, np.tile(np.repeat(np.arange(M), rep)[None, :], (plug.shape[0], 1))
    )
    if not fast:
        return _numpy_fallback(hidden, logw, cumA, plug), None

    nbands = _decide_bands(cumA, logw)
    if nbands not in _prog_cache:
        _prog_cache[nbands] = _build_program(nbands)
    nc = _prog_cache[nbands]

    lt_np = _build_lt(nbands, cumA, logw)

    in_maps = []
    for c in range(NCORES):
        b, q = divmod(c, NQ)
        xq = hidden[b, :, q * QW:(q + 1) * QW]
        xq = (xq.reshape(NG, GROUP, CHUNK, QW)
                .transpose(0, 2, 1, 3)
                .reshape(NG * CHUNK, GROUP * QW))
        in_maps.append({
            "x": np.ascontiguousarray(xq.astype(NP_BF16)),
            "lt": lt_np[b],
        })

    res = run_bass_kernel_spmd(nc, in_maps, list(range(NCORES)), trace=trace)
    out = np.empty((B, LFULL, D_MODEL), np.float32)
    out4 = out.reshape(B, M, rep, D_MODEL)
    for c in range(NCORES):
        b, q = divmod(c, NQ)
        yc = np.asarray(res.results[c]["y"])          # (128, C*QW) bf16
        t = (yc.reshape(CHUNK, C, QW)
               .transpose(1, 0, 2)
               .reshape(M, QW)
               .astype(np.float32))
        out4[b, :, :, q * QW:(q + 1) * QW] = t[:, None, :]
    return out, res


def _numpy_fallback(hidden, logw, cumA, plug):
    """Exact CPU path for plug patterns the device program doesn't cover."""
    y = np.zeros((B, M, D_MODEL), np.float32)
    for b in range(B):
        for i in range(C):
            T0 = i * CHUNK
            acc = np.zeros((CHUNK, D_MODEL), np.float64)
            for j in range(i + 1):
                S0 = j * CHUNK
                arg = (cumA[b, T0:T0 + CHUNK][None, :]
                       - cumA[b, S0:S0 + CHUNK][:, None]
                       + logw[b, S0:S0 + CHUNK][:, None])
                if j == i:
                    s_idx = np.arange(CHUNK)
                    arg = np.where(s_idx[:, None] > s_idx[None, :], -np.inf, arg)
                if arg.max() < UFLOW:
                    continue
                LT = np.exp(arg)
                acc += LT.T @ hidden[b, S0:S0 + CHUNK].astype(np.float64)
            y[b, T0:T0 + CHUNK] = acc.astype(np.float32)
    return np.take_along_axis(y, plug[:, :, None].astype(np.int64), axis=1)


def kernel(**inputs) -> np.ndarray:
    out, _ = _run(inputs, trace=False)
    return out
